# revision 30
# baseline (speedup 1.0000x reference)
"""AttentionLSTMDecoder Trainium2 kernel (8-core SPMD), v2.

Sharding: data-parallel over batch B=64 -> 8 graphs/core for the
recurrent part (attention over that core's node segment + 2-layer LSTM),
AllGather of h1 trajectories in 4-step chunks (bf16), vocab-sharded fc
(each core computes a 4096-wide padded vocab slice for all positions).

v2 changes vs baseline:
- all matmul operands bf16 (weights, activations, stationaries).
- LSTM cell uses only tanh (sigmoid(x) = (1+tanh(x/2))/2, with the 0.5
  gate scales and a doubled-h state folded into the weights host-side)
  -> sigmoid/exp ACT-table thrash eliminated (exp+tanh share one table).
- gate PSUM as [8,512] quarters, double-buffered -> no WAR stalls, the
  W_hh1 part of layer-1 gates runs during the layer-0 cell.
- b_a folded into the mask row host-side (scores += np@b_a).
- fc restructured: hA chunks as stationary, W_fc as 512-wide moving,
  interleaved into recurrence idle slots, bf16 output, bias on host.
"""

import math

import numpy as np

B, T, H, E, V, NTOT = 64, 20, 512, 512, 32000, 8192
NCORES = 8
BL = B // NCORES          # 8 graphs per core
POS = T * BL              # 160 positions per core
VSH = V // NCORES         # 4000 vocab rows per core
VPAD = 4096               # padded vocab shard
G4 = 4 * H                # 2048 gate width
NEG = -40.0               # mask bias for off-segment scores
NCH = (T + 3) // 4        # AllGather chunks (4 steps each)
NROW = NCH * 2            # fc output rows of 128 positions

_COMPILED = {}


def _build(n_pad, use_b0, use_b1):
    import concourse.bacc as bacc
    import concourse.mybir as mybir
    import concourse.tile as tile
    from concourse.alu_op_type import AluOpType
    from contextlib import ExitStack

    f32 = mybir.dt.float32
    bf16 = mybir.dt.bfloat16
    AF = mybir.ActivationFunctionType
    ADD, MULT = AluOpType.add, AluOpType.mult

    nk = n_pad // 128         # node K-tiles
    nck = (n_pad + 511) // 512  # score column chunks
    nc = bacc.Bacc("TRN2", target_bir_lowering=False, debug=False,
                   num_devices=NCORES)

    D = {}
    def din(name, shape, dt=bf16):
        D[name] = nc.dram_tensor(name, shape, dt, kind="ExternalInput").ap()
        return D[name]

    nfT = din("nfT", [128, 5, n_pad])          # [NF.T; ones-row; 0] blocks
    wcT = din("wcT", [128, 5, 512])            # [W_c.T; b_c; 0] blocks
    wcaT = din("wcaT", [128, 5, 512])    # [(W_c.T@W_a)/2; (b_c@W_a)/2] blocks
    msk = din("msk", [128, n_pad])       # mask rhs block (incl np@b_a fold)
    mi8 = din("mi8", [128, 8])           # mask lhsT block (I8 + ones row)
    i8b = din("i8b", [8, 8])             # identity (bf16)
    embT = din("embT", [128, 4, POS])          # emb.T blocks, cols t*8+b
    wembT = din("wembT", [128, 4, G4])         # W_ih0[:, :512].T blocks scaled
    b0c = din("b0c", [128, G4], f32)     # b0 broadcast (only if used)
    w0T = din("w0T", [128, 8, G4])             # [W_ctx.T; W_hh0.T] scaled
    w1T = din("w1T", [128, 8, G4])             # [W_ih1.T; W_hh1.T] scaled
    b1r = din("b1r", [8, G4])            # b1 rows (only if used)
    gfT = din("gfT", [128, 4, 8])              # 2*graph_features.T blocks
    wfcb = din("wfcb", [128, 4, VPAD])         # W_fc.T/2 shard blocks
    out_d = nc.dram_tensor("out", [NROW, 128, VPAD], bf16,
                           kind="ExternalOutput").ap()

    with tile.TileContext(nc) as tc, ExitStack() as ctx:
        res = ctx.enter_context(tc.tile_pool(name="res", bufs=1))
        dram = ctx.enter_context(tc.tile_pool(name="dram", bufs=1, space="DRAM"))
        drsh = ctx.enter_context(tc.tile_pool(name="drsh", bufs=1, space="DRAM"))

        npT = res.tile([128, 5, n_pad], bf16, tag="npT")   # [NPa.T blocks; mask]
        npB = res.tile([128, nk, 512], bf16, tag="npB")    # NP node-major blocks
        i8bs = res.tile([8, 8], bf16, tag="i8bs")
        msT = res.tile([128, 8], bf16, tag="msT")
        hall = res.tile([128, 4, POS], bf16, tag="hall")
        x0T = res.tile([128, 8, 8], bf16, tag="x0T")       # [ctx.T | H0.T]
        x1T = res.tile([128, 8, 8], bf16, tag="x1T")       # [H0n.T | H1.T]
        c0s = res.tile([8, H], f32, tag="c0s")
        c1s = res.tile([8, H], f32, tag="c1s")
        w0s = res.tile([128, 8, G4], bf16, tag="w0s")
        w1s = res.tile([128, 8, G4], bf16, tag="w1s")
        wfcs = res.tile([128, 4, VPAD], bf16, tag="wfcs")
        hA = [res.tile([128, 4, 256], bf16, tag=f"hA{ch}", name=f"hA{ch}")
              for ch in range(NCH)]
        b1s = res.tile([8, G4], bf16, tag="b1s") if use_b1 else None

        nc.sync.dma_start(i8bs[:], i8b[:])
        nc.sync.dma_start(msT[:], mi8[:])
        nc.sync.dma_start(npT[:, 4, :], msk[:])
        nc.sync.dma_start(x0T[:, 4:8, :], gfT[:])
        nc.sync.dma_start(x1T[:, 4:8, :], gfT[:])
        nc.scalar.dma_start(w0s[:], w0T[:])
        nc.scalar.dma_start(w1s[:], w1T[:])
        nc.scalar.dma_start(wfcs[:], wfcb[:])
        nc.gpsimd.memset(c0s[:], 0.0)
        nc.gpsimd.memset(c1s[:], 0.0)
        if use_b1:
            nc.sync.dma_start(b1s[:], b1r[:])

        eg_dram = dram.tile([POS, G4], bf16)
        ag_ins = [dram.tile([512, 32], bf16, tag=f"agi{i}", name=f"agi{i}")
                  for i in range(NCH)]
        ag_outs = [drsh.tile([NCORES * 512, 32], bf16,
                             addr_space="Shared", tag=f"ago{i}",
                             name=f"ago{i}")
                   for i in range(NCH)]

        # ---------------- phase A: NP.T (scores side), NP, EG0 ----------
        with tc.tile_pool(name="pha", bufs=1, side="right") as pha, \
             tc.tile_pool(name="phap", bufs=1, space="PSUM") as phap:
            nfs = pha.tile([128, 5, n_pad], bf16, tag="nfs")
            wcs = pha.tile([128, 5, 512], bf16, tag="wcs")
            was = pha.tile([128, 5, 512], bf16, tag="was")
            nc.sync.dma_start(nfs[:], nfT[:])
            nc.sync.dma_start(wcs[:], wcT[:])
            nc.sync.dma_start(was[:], wcaT[:])

            # NPa.T chunk mt = sum_kt was[:,kt,mt-chunk].T @ nfs[:,kt,:]
            for mt in range(4):
                p = phap.tile([128, n_pad], f32, tag="pa")
                for kt in range(5):
                    lt = was[:, kt, mt * 128:(mt + 1) * 128]
                    for c0 in range(0, n_pad, 512):
                        cw = min(512, n_pad - c0)
                        nc.tensor.matmul(
                            p[:, c0:c0 + cw], lt,
                            nfs[:, kt, c0:c0 + cw],
                            start=(kt == 0), stop=(kt == 4))
                nc.scalar.copy(npT[:, mt, :], p[:])

            # NP block j = sum_kt nfs[:,kt,j-chunk].T @ wcs[:,kt,:]
            for j in range(nk):
                p = phap.tile([128, 512], f32, tag="pb")
                for kt in range(5):
                    nc.tensor.matmul(
                        p[:], nfs[:, kt, j * 128:(j + 1) * 128],
                        wcs[:, kt, :], start=(kt == 0), stop=(kt == 4))
                nc.scalar.copy(npB[:, j, :], p[:])

            # EG0 [POS, 2048] = embT.T @ wembT (+ b0)
            ems = pha.tile([128, 4, POS], bf16, tag="ems")
            nc.sync.dma_start(ems[:], embT[:])
            if use_b0:
                b0s = pha.tile([128, G4], f32, tag="b0s")
                nc.sync.dma_start(b0s[:], b0c[:])
            for mc in range(0, POS, 128):
                mw = min(128, POS - mc)
                p = phap.tile([128, G4], f32, tag="pc")
                for c0 in range(0, G4, 512):
                    wes = pha.tile([128, 4, 512], bf16, tag="wes", bufs=2)
                    nc.sync.dma_start(wes[:], wembT[:, :, c0:c0 + 512])
                    for kt in range(4):
                        nc.tensor.matmul(
                            p[:mw, c0:c0 + 512],
                            ems[:, kt, mc:mc + mw],
                            wes[:, kt, :],
                            start=(kt == 0), stop=(kt == 3))
                for h0_ in (0, 1024):
                    eo = pha.tile([128, 1024], bf16, tag="eo")
                    if use_b0:
                        nc.vector.tensor_add(eo[:mw, :], p[:mw, h0_:h0_ + 1024],
                                             b0s[:mw, h0_:h0_ + 1024])
                    else:
                        nc.scalar.copy(eo[:mw, :], p[:mw, h0_:h0_ + 1024])
                    nc.sync.dma_start(eg_dram[mc:mc + mw, h0_:h0_ + 1024],
                                      eo[:mw, :])

        # ---------------- recurrence + interleaved fc ----------------
        fc_row = [0]      # next output row (ch*2+pc), 0..NROW-1
        fc_vc = [0]       # next vocab chunk within row, 0..7
        fc_cur = [None]   # current fco tile

        with tc.tile_pool(name="stepp", bufs=1) as stepp, \
             tc.tile_pool(name="egp", bufs=2) as egp, \
             tc.tile_pool(name="fco", bufs=2) as fco, \
             tc.tile_pool(name="gp", bufs=2, space="PSUM") as gp, \
             tc.tile_pool(name="scp", bufs=2, space="PSUM") as scp, \
             tc.tile_pool(name="sml", bufs=1, space="PSUM") as sml, \
             tc.tile_pool(name="fcp", bufs=3, space="PSUM") as fcp:

            def fc_unit():
                """One (row, vc) fc unit: 4 matmuls + copy; DMA on row end."""
                row, vc = fc_row[0], fc_vc[0]
                ch, pc = divmod(row, 2)
                if vc == 0:
                    fc_cur[0] = fco.tile([128, VPAD], bf16, tag="fcr",
                                         name=f"fcr{row}")
                p = fcp.tile([128, 512], f32, tag="fc")
                for kt in range(4):
                    nc.tensor.matmul(p[:], hA[ch][:, kt, pc * 128:(pc + 1) * 128],
                                     wfcs[:, kt, vc * 512:(vc + 1) * 512],
                                     start=(kt == 0), stop=(kt == 3))
                nc.scalar.copy(fc_cur[0][:, vc * 512:(vc + 1) * 512], p[:])
                fc_vc[0] += 1
                if fc_vc[0] == 8:
                    nc.sync.dma_start(out_d[row], fc_cur[0][:])
                    fc_row[0] += 1
                    fc_vc[0] = 0

            def cell(Tg, cS, dsts, hall_slice=None):
                """LSTM cell from tanh'd gates Tg [8,2048] (t_i|t_f|t_g|t_o
                with i,f,o pre-halved); updates cS (=2c) in place, writes
                the transposed doubled hidden state into dsts."""
                u = stepp.tile([8, 512], f32, tag="u")
                nc.vector.scalar_tensor_tensor(
                    u[:], Tg[:, 512:1024], 1.0, cS[:], ADD, MULT)
                v = stepp.tile([8, 512], f32, tag="v")
                nc.vector.scalar_tensor_tensor(
                    v[:], Tg[:, 0:512], 1.0, Tg[:, 1024:1536], ADD, MULT)
                nc.vector.scalar_tensor_tensor(
                    cS[:], u[:], 0.5, v[:], MULT, ADD)
                tch = stepp.tile([8, 512], f32, tag="tch")
                nc.scalar.activation(tch[:], cS[:], AF.Tanh, scale=0.5)
                hn = stepp.tile([8, 512], bf16, tag="hn")
                nc.vector.scalar_tensor_tensor(
                    hn[:], Tg[:, 1536:2048], 1.0, tch[:], ADD, MULT)
                tp = sml.tile([128, 96], bf16, tag="tp")
                for j in range(4):
                    nc.tensor.transpose(tp[:, j * 8:(j + 1) * 8],
                                        hn[:, j * 128:(j + 1) * 128], i8bs[:])
                tpv = tp[:, 0:32].rearrange("p (a b) -> p a b", a=4)
                for dst in dsts:
                    nc.vector.tensor_copy(dst, tpv)
                if hall_slice is not None:
                    nc.vector.tensor_copy(hall_slice, tpv)

            def ha_load(ch):
                # deferred: emitted one step before first fc use, when the
                # AllGather is long done (no scalar-queue head-of-line stall)
                for c in range(NCORES):
                    nc.scalar.dma_start(
                        hA[ch][:, :, c * 32:(c + 1) * 32],
                        ag_outs[ch][c * 512:(c + 1) * 512].rearrange(
                            "(a p) n -> p a n", p=128))

            for t in range(T):
                if t >= 5 and (t - 5) % 4 == 0 and (t - 5) // 4 < NCH - 1:
                    ha_load((t - 5) // 4)
                eg = egp.tile([8, G4], bf16, tag="eg")
                nc.sync.dma_start(eg[:], eg_dram[t * 8:(t + 1) * 8, :])

                # scores S.T [8, n_pad] = H1/2 @ NPa.T + mask, in 512-chunks
                Et = stepp.tile([8, n_pad], bf16, tag="Et")
                dp = stepp.tile([8, 4], f32, tag="dp")
                for c in range(nck):
                    c0 = c * 512
                    cw = min(512, n_pad - c0)
                    sc = scp.tile([8, 512], f32, tag="sc")
                    for kt in (4, 0, 1, 2, 3):
                        lt = msT[:] if kt == 4 else x1T[:, 4 + kt, :]
                        nc.tensor.matmul(sc[:, 0:cw], lt,
                                         npT[:, kt, c0:c0 + cw],
                                         start=(kt == 4), stop=(kt == 3))
                    nc.scalar.activation(Et[:, c0:c0 + cw], sc[:, 0:cw],
                                         AF.Exp, accum_out=dp[:, c:c + 1])
                den = stepp.tile([8, 1], f32, tag="den")
                if nck == 1:
                    den = dp[:, 0:1]
                else:
                    nc.vector.tensor_add(den[:], dp[:, 0:1], dp[:, 1:2])
                    for c in range(2, nck):
                        nc.vector.tensor_add(den[:], den[:], dp[:, c:c + 1])
                r8 = stepp.tile([8, 1], f32, tag="r8")
                nc.vector.reciprocal(r8[:], den[:])

                # E.T via PE transposes
                etP = sml.tile([128, 96], bf16, tag="tp")
                for j in range(nk):
                    nc.tensor.transpose(etP[:, j * 8:(j + 1) * 8],
                                        Et[:, j * 128:(j + 1) * 128], i8bs[:])
                etT = stepp.tile([128, nk, 8], bf16, tag="etT")
                nc.vector.tensor_copy(
                    etT[:], etP[:, 0:nk * 8].rearrange("p (a b) -> p a b", a=nk))

                # ctx [8, 512] = E @ NP, scaled by 1/den on copy-out
                ctxP = scp.tile([8, 512], f32, tag="sc")
                for j in range(nk):
                    nc.tensor.matmul(ctxP[:], etT[:, j, :], npB[:, j, :],
                                     start=(j == 0), stop=(j == nk - 1))
                ctxS = stepp.tile([8, 512], bf16, tag="ctxS")
                nc.scalar.activation(ctxS[:], ctxP[:], AF.Copy, scale=r8[:])

                # ctx.T -> x0T[:, 0:4, :]
                ctP = sml.tile([128, 96], bf16, tag="tp")
                for j in range(4):
                    nc.tensor.transpose(ctP[:, j * 8:(j + 1) * 8],
                                        ctxS[:, j * 128:(j + 1) * 128], i8bs[:])
                nc.vector.tensor_copy(
                    x0T[:, 0:4, :],
                    ctP[:, 0:32].rearrange("p (a b) -> p a b", a=4))

                # gates0 in [8,512] quarters: sum_kt x0T.T @ w0 + EG0[t]
                Tg0 = stepp.tile([8, G4], f32, tag="Tg0")
                for q in range(4):
                    qs = q * 512
                    g = gp.tile([8, 512], f32, tag="g")
                    for kt in range(8):
                        nc.tensor.matmul(g[:], x0T[:, kt, :],
                                         w0s[:, kt, qs:qs + 512],
                                         start=(kt == 0), stop=False)
                    nc.tensor.matmul(g[:], i8bs[:], eg[:, qs:qs + 512],
                                     start=False, stop=True)
                    nc.scalar.activation(Tg0[:, qs:qs + 512], g[:], AF.Tanh)
                cell(Tg0, c0s, [x1T[:, 0:4, :], x0T[:, 4:8, :]])

                # gates1: h1-parts of q0/q1 early (overlap cell0), then close
                Tg1 = stepp.tile([8, G4], f32, tag="Tg1")
                g1q = [None] * 4
                def g1_open(q):
                    g = gp.tile([8, 512], f32, tag="g")
                    g1q[q] = g
                    for kt in range(4, 8):
                        nc.tensor.matmul(g[:], x1T[:, kt, :],
                                         w1s[:, kt, q * 512:q * 512 + 512],
                                         start=(kt == 4), stop=False)
                def g1_close(q):
                    g = g1q[q]
                    qs = q * 512
                    for kt in range(4):
                        nc.tensor.matmul(g[:], x1T[:, kt, :],
                                         w1s[:, kt, qs:qs + 512],
                                         start=False,
                                         stop=(kt == 3 and not use_b1))
                    if use_b1:
                        nc.tensor.matmul(g[:], i8bs[:], b1s[:, qs:qs + 512],
                                         start=False, stop=True)
                    nc.scalar.activation(Tg1[:, qs:qs + 512], g[:], AF.Tanh)
                g1_open(0)
                g1_open(1)
                g1_close(0)
                g1_close(1)
                g1_open(2)
                g1_close(2)
                g1_open(3)
                g1_close(3)

                # interleave fc work into the cell1 window
                avail_rows = 0 if t < 6 else min(NROW, 2 * ((t - 6) // 4 + 1))
                budget = 2
                while budget > 0 and fc_row[0] < avail_rows:
                    fc_unit()
                    budget -= 1

                cell(Tg1, c1s, [x1T[:, 4:8, :]],
                     hall_slice=hall[:, :, t * 8:(t + 1) * 8])

                if t % 4 == 3:
                    ch = t // 4
                    agi = ag_ins[ch]
                    nc.sync.dma_start(
                        agi[:].rearrange("(a p) n -> p a n", p=128),
                        hall[:, :, ch * 32:(ch + 1) * 32])
                    nc.gpsimd.collective_compute(
                        "AllGather", mybir.AluOpType.bypass,
                        replica_groups=[list(range(NCORES))],
                        ins=[agi.opt()], outs=[ag_outs[ch].opt()])

            # ---------------- fc tail ----------------
            ha_load(NCH - 1)
            while fc_row[0] < NROW:
                fc_unit()

    nc.compile()
    return nc


def _prep(inputs, n_pad):
    import ml_dtypes
    bf = ml_dtypes.bfloat16
    gf = np.ascontiguousarray(np.asarray(inputs["graph_features"], np.float32))
    nf = np.ascontiguousarray(np.asarray(inputs["node_features"], np.float32))
    emb = np.asarray(inputs["embedding"], np.float32)
    W_a = np.asarray(inputs["W_a"], np.float32)
    b_a = np.asarray(inputs["b_a"], np.float32)
    W_c = np.asarray(inputs["W_c"], np.float32)
    b_c = np.asarray(inputs["b_c"], np.float32)
    W_ih0 = np.asarray(inputs["W_ih0"], np.float32)
    W_hh0 = np.asarray(inputs["W_hh0"], np.float32)
    b0 = np.asarray(inputs["b_ih0"], np.float32) + np.asarray(inputs["b_hh0"], np.float32)
    W_ih1 = np.asarray(inputs["W_ih1"], np.float32)
    W_hh1 = np.asarray(inputs["W_hh1"], np.float32)
    b1 = np.asarray(inputs["b_ih1"], np.float32) + np.asarray(inputs["b_hh1"], np.float32)
    W_fc = np.asarray(inputs["W_fc"], np.float32)
    bidx = np.asarray(inputs["batch_idx"]).astype(np.int64)
    caps = np.asarray(inputs["captions"]).astype(np.int64)

    # gate scale: i,f,o gates halved (sigmoid-via-tanh); g full.
    gsc = np.ones((G4,), np.float32) * 0.5
    gsc[2 * H:3 * H] = 1.0        # g gate (order i,f,g,o)
    # h-doubling: consumers of h scale by 0.5
    w0 = np.concatenate([W_ih0[:, 512:].T * gsc[None, :],
                         W_hh0.T * (0.5 * gsc)[None, :]], 0)
    w1 = np.concatenate([W_ih1.T * (0.5 * gsc)[None, :],
                         W_hh1.T * (0.5 * gsc)[None, :]], 0)
    wemb = W_ih0[:, :512].T * gsc[None, :]
    b0s = b0 * gsc
    b1s = b1 * gsc

    def blocks(a):
        K, N = a.shape
        return np.ascontiguousarray(a.reshape(K // 128, 128, N).transpose(1, 0, 2))

    wcT_full = np.zeros((640, 512), np.float32)
    wcT_full[:512] = W_c.T
    wcT_full[512] = b_c
    wca_full = np.zeros((640, 512), np.float32)
    wca_full[:512] = 0.5 * (W_c.T @ W_a)
    wca_full[512] = 0.5 * (b_c @ W_a)
    i8 = np.eye(8, dtype=np.float32)
    mi8 = np.zeros((128, 8), np.float32)
    mi8[:8, :8] = np.eye(8)
    mi8[8, :] = 1.0
    b0c = np.tile(b0s[None, :], (128, 1)).astype(np.float32)
    b1r = np.tile(b1s[None, :], (8, 1))
    use_b0 = bool(np.any(b0 != 0))
    use_b1 = bool(np.any(b1 != 0))
    sb_ba = (nf @ W_c.T + b_c) @ b_a      # per-node b_a fold for scores

    maps = []
    for k in range(NCORES):
        sel = (bidx >= k * BL) & (bidx < (k + 1) * BL)
        nodes = np.nonzero(sel)[0]
        cnt = len(nodes)
        nfT_full = np.zeros((640, n_pad), np.float32)
        nfT_full[:512, :cnt] = nf[nodes].T
        nfT_full[512, :cnt] = 1.0
        lb = bidx[nodes] - k * BL
        msk = np.zeros((128, n_pad), np.float32)
        msk[8, :] = NEG
        msk[8, :cnt] += sb_ba[nodes]
        msk[lb, np.arange(cnt)] = -NEG
        e = emb[caps[k * BL:(k + 1) * BL]]             # [8, T, E]
        embT_full = np.ascontiguousarray(e.transpose(2, 1, 0).reshape(E, POS))
        wfc = np.zeros((VPAD, H), np.float32)
        wfc[:VSH] = 0.5 * W_fc[k * VSH:(k + 1) * VSH]
        wfcb = blocks(np.ascontiguousarray(wfc.T))     # [128, 4, VPAD]
        m = {
            "nfT": blocks(nfT_full).astype(bf),
            "wcT": blocks(wcT_full).astype(bf),
            "wcaT": blocks(wca_full).astype(bf),
            "msk": msk.astype(bf), "mi8": mi8.astype(bf),
            "i8b": i8.astype(bf),
            "embT": blocks(embT_full).astype(bf),
            "wembT": blocks(wemb).astype(bf), "b0c": b0c,
            "w0T": blocks(w0).astype(bf), "w1T": blocks(w1).astype(bf),
            "b1r": b1r.astype(bf),
            "gfT": blocks(np.ascontiguousarray(
                2.0 * gf[k * BL:(k + 1) * BL].T)).astype(bf),
            "wfcb": wfcb.astype(bf),
        }
        maps.append(m)
    return maps, use_b0, use_b1


def kernel(**inputs) -> np.ndarray:
    from concourse.bass_utils import run_bass_kernel_spmd

    bidx = np.asarray(inputs["batch_idx"]).astype(np.int64)
    counts = np.bincount(bidx // BL, minlength=NCORES)
    n_pad = max(256, int(math.ceil(counts.max() / 128.0)) * 128)
    maps, use_b0, use_b1 = _prep(inputs, n_pad)
    key = (n_pad, use_b0, use_b1)
    if key not in _COMPILED:
        _COMPILED[key] = _build(n_pad, use_b0, use_b1)
    res = run_bass_kernel_spmd(_COMPILED[key], maps,
                               core_ids=list(range(NCORES)))
    b_fc = np.asarray(inputs["b_fc"], np.float32)
    out = np.empty((B, T, V), np.float32)
    for k in range(NCORES):
        o = np.asarray(res.results[k]["out"]).astype(np.float32)
        # [ch, pc, c4, dt, b, v] -> [pc, c4, b, ch, dt, v]
        o = o.reshape(NCH, 2, 4, 4, 8, VPAD).transpose(1, 2, 4, 0, 3, 5)
        out[:, :, k * VSH:(k + 1) * VSH] = o.reshape(B, T, VPAD)[:, :, :VSH]
    if np.any(b_fc != 0):
        out += b_fc[None, None, :]
    return out


# revision 31
# speedup vs baseline: 1.0400x; 1.0400x over previous
"""AttentionLSTMDecoder Trainium2 kernel (8-core SPMD), v2.

Sharding: data-parallel over batch B=64 -> 8 graphs/core for the
recurrent part (attention over that core's node segment + 2-layer LSTM),
AllGather of h1 trajectories in 4-step chunks (bf16), vocab-sharded fc
(each core computes a 4096-wide padded vocab slice for all positions).

v2 changes vs baseline:
- all matmul operands bf16 (weights, activations, stationaries).
- LSTM cell uses only tanh (sigmoid(x) = (1+tanh(x/2))/2, with the 0.5
  gate scales and a doubled-h state folded into the weights host-side)
  -> sigmoid/exp ACT-table thrash eliminated (exp+tanh share one table).
- gate PSUM as [8,512] quarters, double-buffered -> no WAR stalls, the
  W_hh1 part of layer-1 gates runs during the layer-0 cell.
- b_a folded into the mask row host-side (scores += np@b_a).
- fc restructured: hA chunks as stationary, W_fc as 512-wide moving,
  interleaved into recurrence idle slots, bf16 output, bias on host.
"""

import math

import numpy as np

B, T, H, E, V, NTOT = 64, 20, 512, 512, 32000, 8192
NCORES = 8
BL = B // NCORES          # 8 graphs per core
POS = T * BL              # 160 positions per core
VSH = V // NCORES         # 4000 vocab rows per core
VPAD = 4096               # padded vocab shard
G4 = 4 * H                # 2048 gate width
NEG = -40.0               # mask bias for off-segment scores
NCH = (T + 3) // 4        # AllGather chunks (4 steps each)
NROW = NCH * 2            # fc output rows of 128 positions

_COMPILED = {}


def _build(n_pad, use_b0, use_b1):
    import concourse.bacc as bacc
    import concourse.mybir as mybir
    import concourse.tile as tile
    from concourse.alu_op_type import AluOpType
    from contextlib import ExitStack

    f32 = mybir.dt.float32
    bf16 = mybir.dt.bfloat16
    AF = mybir.ActivationFunctionType
    ADD, MULT = AluOpType.add, AluOpType.mult

    nk = n_pad // 128         # node K-tiles
    nck = (n_pad + 511) // 512  # score column chunks
    nc = bacc.Bacc("TRN2", target_bir_lowering=False, debug=False,
                   num_devices=NCORES)

    D = {}
    def din(name, shape, dt=bf16):
        D[name] = nc.dram_tensor(name, shape, dt, kind="ExternalInput").ap()
        return D[name]

    nfT = din("nfT", [128, 5, n_pad])          # [NF.T; ones-row; 0] blocks
    wcT = din("wcT", [128, 5, 512])            # [W_c.T; b_c; 0] blocks
    wcaT = din("wcaT", [128, 5, 512])    # [(W_c.T@W_a)/2; (b_c@W_a)/2] blocks
    msk = din("msk", [128, n_pad])       # mask rhs block (incl np@b_a fold)
    mi8 = din("mi8", [128, 8])           # mask lhsT block (I8 + ones row)
    i8b = din("i8b", [8, 8])             # identity (bf16)
    embT = din("embT", [128, 4, POS])          # emb.T blocks, cols t*8+b
    wembT = din("wembT", [128, 4, G4])         # W_ih0[:, :512].T blocks scaled
    b0c = din("b0c", [128, G4], f32)     # b0 broadcast (only if used)
    w0T = din("w0T", [128, 8, G4])             # [W_ctx.T; W_hh0.T] scaled
    w1T = din("w1T", [128, 8, G4])             # [W_ih1.T; W_hh1.T] scaled
    b1r = din("b1r", [8, G4])            # b1 rows (only if used)
    gfT = din("gfT", [128, 4, 8])              # 2*graph_features.T blocks
    wfcb = din("wfcb", [128, 4, VPAD])         # W_fc.T/2 shard blocks
    out_d = nc.dram_tensor("out", [NROW, 128, VPAD], bf16,
                           kind="ExternalOutput").ap()

    with tile.TileContext(nc) as tc, ExitStack() as ctx:
        res = ctx.enter_context(tc.tile_pool(name="res", bufs=1))
        dram = ctx.enter_context(tc.tile_pool(name="dram", bufs=1, space="DRAM"))
        drsh = ctx.enter_context(tc.tile_pool(name="drsh", bufs=1, space="DRAM"))

        npT = res.tile([128, 5, n_pad], bf16, tag="npT")   # [NPa.T blocks; mask]
        npB = res.tile([128, nk, 512], bf16, tag="npB")    # NP node-major blocks
        i8bs = res.tile([8, 8], bf16, tag="i8bs")
        msT = res.tile([128, 8], bf16, tag="msT")
        hall = res.tile([128, 4, POS], bf16, tag="hall")
        x0T = res.tile([128, 8, 8], bf16, tag="x0T")       # [ctx.T | H0.T]
        x1T = res.tile([128, 8, 8], bf16, tag="x1T")       # [H0n.T | H1.T]
        c0s = res.tile([8, H], f32, tag="c0s")
        c1s = res.tile([8, H], f32, tag="c1s")
        w0s = res.tile([128, 8, G4], bf16, tag="w0s")
        w1s = res.tile([128, 8, G4], bf16, tag="w1s")
        wfcs = res.tile([128, 4, VPAD], bf16, tag="wfcs")
        hA = [res.tile([128, 4, 256], bf16, tag=f"hA{ch}", name=f"hA{ch}")
              for ch in range(NCH)]
        b1s = res.tile([8, G4], bf16, tag="b1s") if use_b1 else None

        nc.sync.dma_start(i8bs[:], i8b[:])
        nc.sync.dma_start(msT[:], mi8[:])
        nc.sync.dma_start(npT[:, 4, :], msk[:])
        nc.sync.dma_start(x0T[:, 4:8, :], gfT[:])
        nc.sync.dma_start(x1T[:, 4:8, :], gfT[:])
        nc.scalar.dma_start(w0s[:], w0T[:])
        nc.scalar.dma_start(w1s[:], w1T[:])
        nc.scalar.dma_start(wfcs[:], wfcb[:])
        nc.gpsimd.memset(c0s[:], 0.0)
        nc.gpsimd.memset(c1s[:], 0.0)
        if use_b1:
            nc.sync.dma_start(b1s[:], b1r[:])

        eg_dram = dram.tile([POS, G4], bf16)
        ag_ins = [dram.tile([512, 32], bf16, tag=f"agi{i}", name=f"agi{i}")
                  for i in range(NCH)]
        ag_outs = [drsh.tile([NCORES * 512, 32], bf16,
                             addr_space="Shared", tag=f"ago{i}",
                             name=f"ago{i}")
                   for i in range(NCH)]

        # ---------------- phase A: NP.T (scores side), NP, EG0 ----------
        with tc.tile_pool(name="pha", bufs=1, side="right") as pha, \
             tc.tile_pool(name="phap", bufs=1, space="PSUM") as phap:
            nfs = pha.tile([128, 5, n_pad], bf16, tag="nfs")
            wcs = pha.tile([128, 5, 512], bf16, tag="wcs")
            was = pha.tile([128, 5, 512], bf16, tag="was")
            nc.sync.dma_start(nfs[:], nfT[:])
            nc.sync.dma_start(wcs[:], wcT[:])
            nc.sync.dma_start(was[:], wcaT[:])

            # NPa.T chunk mt = sum_kt was[:,kt,mt-chunk].T @ nfs[:,kt,:]
            for mt in range(4):
                p = phap.tile([128, n_pad], f32, tag="pa")
                for kt in range(5):
                    lt = was[:, kt, mt * 128:(mt + 1) * 128]
                    for c0 in range(0, n_pad, 512):
                        cw = min(512, n_pad - c0)
                        nc.tensor.matmul(
                            p[:, c0:c0 + cw], lt,
                            nfs[:, kt, c0:c0 + cw],
                            start=(kt == 0), stop=(kt == 4))
                nc.scalar.copy(npT[:, mt, :], p[:])

            # NP block j = sum_kt nfs[:,kt,j-chunk].T @ wcs[:,kt,:]
            for j in range(nk):
                p = phap.tile([128, 512], f32, tag="pb")
                for kt in range(5):
                    nc.tensor.matmul(
                        p[:], nfs[:, kt, j * 128:(j + 1) * 128],
                        wcs[:, kt, :], start=(kt == 0), stop=(kt == 4))
                nc.scalar.copy(npB[:, j, :], p[:])

            # EG0 [POS, 2048] = embT.T @ wembT (+ b0)
            ems = pha.tile([128, 4, POS], bf16, tag="ems")
            nc.sync.dma_start(ems[:], embT[:])
            if use_b0:
                b0s = pha.tile([128, G4], f32, tag="b0s")
                nc.sync.dma_start(b0s[:], b0c[:])
            for mc in range(0, POS, 128):
                mw = min(128, POS - mc)
                p = phap.tile([128, G4], f32, tag="pc")
                for c0 in range(0, G4, 512):
                    wes = pha.tile([128, 4, 512], bf16, tag="wes", bufs=2)
                    nc.sync.dma_start(wes[:], wembT[:, :, c0:c0 + 512])
                    for kt in range(4):
                        nc.tensor.matmul(
                            p[:mw, c0:c0 + 512],
                            ems[:, kt, mc:mc + mw],
                            wes[:, kt, :],
                            start=(kt == 0), stop=(kt == 3))
                for h0_ in (0, 1024):
                    eo = pha.tile([128, 1024], bf16, tag="eo")
                    if use_b0:
                        nc.vector.tensor_add(eo[:mw, :], p[:mw, h0_:h0_ + 1024],
                                             b0s[:mw, h0_:h0_ + 1024])
                    else:
                        nc.scalar.copy(eo[:mw, :], p[:mw, h0_:h0_ + 1024])
                    nc.sync.dma_start(eg_dram[mc:mc + mw, h0_:h0_ + 1024],
                                      eo[:mw, :])

        # ---------------- recurrence + interleaved fc ----------------
        fc_row = [0]      # next output row (ch*2+pc), 0..NROW-1
        fc_vc = [0]       # next vocab chunk within row, 0..7
        fc_cur = [None]   # current fco tile

        with tc.tile_pool(name="stepp", bufs=1) as stepp, \
             tc.tile_pool(name="egp", bufs=2) as egp, \
             tc.tile_pool(name="fco", bufs=2) as fco, \
             tc.tile_pool(name="gp", bufs=2, space="PSUM") as gp, \
             tc.tile_pool(name="scp", bufs=2, space="PSUM") as scp, \
             tc.tile_pool(name="sml", bufs=1, space="PSUM") as sml, \
             tc.tile_pool(name="fcp", bufs=3, space="PSUM") as fcp:

            def fc_unit():
                """One (row, vc) fc unit: 4 matmuls + copy; DMA on row end."""
                row, vc = fc_row[0], fc_vc[0]
                ch, pc = divmod(row, 2)
                if vc == 0:
                    fc_cur[0] = fco.tile([128, VPAD], bf16, tag="fcr",
                                         name=f"fcr{row}")
                p = fcp.tile([128, 512], f32, tag="fc")
                for kt in range(4):
                    nc.tensor.matmul(p[:], hA[ch][:, kt, pc * 128:(pc + 1) * 128],
                                     wfcs[:, kt, vc * 512:(vc + 1) * 512],
                                     start=(kt == 0), stop=(kt == 3))
                nc.scalar.copy(fc_cur[0][:, vc * 512:(vc + 1) * 512], p[:])
                fc_vc[0] += 1
                if fc_vc[0] == 8:
                    nc.sync.dma_start(out_d[row], fc_cur[0][:])
                    fc_row[0] += 1
                    fc_vc[0] = 0

            def cell(Tg, cS, dsts, hall_slice=None):
                """LSTM cell from tanh'd gates Tg [8,2048] (t_i|t_f|t_g|t_o
                with i,f,o pre-halved); updates cS (=2c) in place, writes
                the transposed doubled hidden state into dsts."""
                u = stepp.tile([8, 512], f32, tag="u")
                nc.vector.scalar_tensor_tensor(
                    u[:], Tg[:, 512:1024], 1.0, cS[:], ADD, MULT)
                v = stepp.tile([8, 512], f32, tag="v")
                nc.vector.scalar_tensor_tensor(
                    v[:], Tg[:, 0:512], 1.0, Tg[:, 1024:1536], ADD, MULT)
                nc.vector.scalar_tensor_tensor(
                    cS[:], u[:], 0.5, v[:], MULT, ADD)
                tch = stepp.tile([8, 512], f32, tag="tch")
                nc.scalar.activation(tch[:], cS[:], AF.Tanh, scale=0.5)
                hn = stepp.tile([8, 512], bf16, tag="hn")
                nc.vector.scalar_tensor_tensor(
                    hn[:], Tg[:, 1536:2048], 1.0, tch[:], ADD, MULT)
                tp = sml.tile([128, 96], bf16, tag="tp")
                for j in range(4):
                    nc.tensor.transpose(tp[:, j * 8:(j + 1) * 8],
                                        hn[:, j * 128:(j + 1) * 128], i8bs[:])
                tpv = tp[:, 0:32].rearrange("p (a b) -> p a b", a=4)
                for dst in dsts:
                    nc.vector.tensor_copy(dst, tpv)
                if hall_slice is not None:
                    nc.vector.tensor_copy(hall_slice, tpv)

            for t in range(T):
                eg = egp.tile([8, G4], bf16, tag="eg")
                nc.sync.dma_start(eg[:], eg_dram[t * 8:(t + 1) * 8, :])

                # scores S.T [8, n_pad] = H1/2 @ NPa.T + mask, in 512-chunks
                Et = stepp.tile([8, n_pad], bf16, tag="Et")
                dp = stepp.tile([8, 4], f32, tag="dp")
                for c in range(nck):
                    c0 = c * 512
                    cw = min(512, n_pad - c0)
                    sc = scp.tile([8, 512], f32, tag="sc")
                    for kt in (4, 0, 1, 2, 3):
                        lt = msT[:] if kt == 4 else x1T[:, 4 + kt, :]
                        nc.tensor.matmul(sc[:, 0:cw], lt,
                                         npT[:, kt, c0:c0 + cw],
                                         start=(kt == 4), stop=(kt == 3))
                    nc.scalar.activation(Et[:, c0:c0 + cw], sc[:, 0:cw],
                                         AF.Exp, accum_out=dp[:, c:c + 1])
                den = stepp.tile([8, 1], f32, tag="den")
                if nck == 1:
                    den = dp[:, 0:1]
                else:
                    nc.vector.tensor_add(den[:], dp[:, 0:1], dp[:, 1:2])
                    for c in range(2, nck):
                        nc.vector.tensor_add(den[:], den[:], dp[:, c:c + 1])
                r8 = stepp.tile([8, 1], f32, tag="r8")
                nc.vector.reciprocal(r8[:], den[:])

                # E.T via PE transposes
                etP = sml.tile([128, 96], bf16, tag="tp")
                for j in range(nk):
                    nc.tensor.transpose(etP[:, j * 8:(j + 1) * 8],
                                        Et[:, j * 128:(j + 1) * 128], i8bs[:])
                etT = stepp.tile([128, nk, 8], bf16, tag="etT")
                nc.vector.tensor_copy(
                    etT[:], etP[:, 0:nk * 8].rearrange("p (a b) -> p a b", a=nk))

                # ctx [8, 512] = E @ NP, scaled by 1/den on copy-out
                ctxP = scp.tile([8, 512], f32, tag="sc")
                for j in range(nk):
                    nc.tensor.matmul(ctxP[:], etT[:, j, :], npB[:, j, :],
                                     start=(j == 0), stop=(j == nk - 1))
                ctxS = stepp.tile([8, 512], bf16, tag="ctxS")
                nc.scalar.activation(ctxS[:], ctxP[:], AF.Copy, scale=r8[:])

                # ctx.T -> x0T[:, 0:4, :]
                ctP = sml.tile([128, 96], bf16, tag="tp")
                for j in range(4):
                    nc.tensor.transpose(ctP[:, j * 8:(j + 1) * 8],
                                        ctxS[:, j * 128:(j + 1) * 128], i8bs[:])
                nc.vector.tensor_copy(
                    x0T[:, 0:4, :],
                    ctP[:, 0:32].rearrange("p (a b) -> p a b", a=4))

                # gates0 in [8,512] quarters: sum_kt x0T.T @ w0 + EG0[t]
                Tg0 = stepp.tile([8, G4], f32, tag="Tg0")
                for q in range(4):
                    qs = q * 512
                    g = gp.tile([8, 512], f32, tag="g")
                    for kt in range(8):
                        nc.tensor.matmul(g[:], x0T[:, kt, :],
                                         w0s[:, kt, qs:qs + 512],
                                         start=(kt == 0), stop=False)
                    nc.tensor.matmul(g[:], i8bs[:], eg[:, qs:qs + 512],
                                     start=False, stop=True)
                    nc.scalar.activation(Tg0[:, qs:qs + 512], g[:], AF.Tanh)
                cell(Tg0, c0s, [x1T[:, 0:4, :], x0T[:, 4:8, :]])

                # gates1: h1-parts of q0/q1 early (overlap cell0), then close
                Tg1 = stepp.tile([8, G4], f32, tag="Tg1")
                g1q = [None] * 4
                def g1_open(q):
                    g = gp.tile([8, 512], f32, tag="g")
                    g1q[q] = g
                    for kt in range(4, 8):
                        nc.tensor.matmul(g[:], x1T[:, kt, :],
                                         w1s[:, kt, q * 512:q * 512 + 512],
                                         start=(kt == 4), stop=False)
                def g1_close(q):
                    g = g1q[q]
                    qs = q * 512
                    for kt in range(4):
                        nc.tensor.matmul(g[:], x1T[:, kt, :],
                                         w1s[:, kt, qs:qs + 512],
                                         start=False,
                                         stop=(kt == 3 and not use_b1))
                    if use_b1:
                        nc.tensor.matmul(g[:], i8bs[:], b1s[:, qs:qs + 512],
                                         start=False, stop=True)
                    nc.scalar.activation(Tg1[:, qs:qs + 512], g[:], AF.Tanh)
                g1_open(0)
                g1_open(1)
                g1_close(0)
                g1_close(1)
                g1_open(2)
                g1_close(2)
                g1_open(3)
                g1_close(3)

                # interleave fc work into the cell1 window
                avail_rows = 0 if t < 6 else min(NROW, 2 * ((t - 6) // 4 + 1))
                budget = 2
                while budget > 0 and fc_row[0] < avail_rows:
                    fc_unit()
                    budget -= 1

                cell(Tg1, c1s, [x1T[:, 4:8, :]],
                     hall_slice=hall[:, :, t * 8:(t + 1) * 8])

                if t % 4 == 3:
                    ch = t // 4
                    agi = ag_ins[ch]
                    nc.sync.dma_start(
                        agi[:].rearrange("(a p) n -> p a n", p=128),
                        hall[:, :, ch * 32:(ch + 1) * 32])
                    nc.gpsimd.collective_compute(
                        "AllGather", mybir.AluOpType.bypass,
                        replica_groups=[list(range(NCORES))],
                        ins=[agi.opt()], outs=[ag_outs[ch].opt()])
                    for c in range(NCORES):
                        nc.scalar.dma_start(
                            hA[ch][:, :, c * 32:(c + 1) * 32],
                            ag_outs[ch][c * 512:(c + 1) * 512].rearrange(
                                "(a p) n -> p a n", p=128))

            # ---------------- fc tail ----------------
            while fc_row[0] < NROW:
                fc_unit()

    nc.compile()
    return nc


def _prep(inputs, n_pad):
    import ml_dtypes
    bf = ml_dtypes.bfloat16
    gf = np.ascontiguousarray(np.asarray(inputs["graph_features"], np.float32))
    nf = np.ascontiguousarray(np.asarray(inputs["node_features"], np.float32))
    emb = np.asarray(inputs["embedding"], np.float32)
    W_a = np.asarray(inputs["W_a"], np.float32)
    b_a = np.asarray(inputs["b_a"], np.float32)
    W_c = np.asarray(inputs["W_c"], np.float32)
    b_c = np.asarray(inputs["b_c"], np.float32)
    W_ih0 = np.asarray(inputs["W_ih0"], np.float32)
    W_hh0 = np.asarray(inputs["W_hh0"], np.float32)
    b0 = np.asarray(inputs["b_ih0"], np.float32) + np.asarray(inputs["b_hh0"], np.float32)
    W_ih1 = np.asarray(inputs["W_ih1"], np.float32)
    W_hh1 = np.asarray(inputs["W_hh1"], np.float32)
    b1 = np.asarray(inputs["b_ih1"], np.float32) + np.asarray(inputs["b_hh1"], np.float32)
    W_fc = np.asarray(inputs["W_fc"], np.float32)
    bidx = np.asarray(inputs["batch_idx"]).astype(np.int64)
    caps = np.asarray(inputs["captions"]).astype(np.int64)

    # gate scale: i,f,o gates halved (sigmoid-via-tanh); g full.
    gsc = np.ones((G4,), np.float32) * 0.5
    gsc[2 * H:3 * H] = 1.0        # g gate (order i,f,g,o)
    # h-doubling: consumers of h scale by 0.5
    w0 = np.concatenate([W_ih0[:, 512:].T * gsc[None, :],
                         W_hh0.T * (0.5 * gsc)[None, :]], 0)
    w1 = np.concatenate([W_ih1.T * (0.5 * gsc)[None, :],
                         W_hh1.T * (0.5 * gsc)[None, :]], 0)
    wemb = W_ih0[:, :512].T * gsc[None, :]
    b0s = b0 * gsc
    b1s = b1 * gsc

    def blocks(a):
        K, N = a.shape
        return np.ascontiguousarray(a.reshape(K // 128, 128, N).transpose(1, 0, 2))

    wcT_full = np.zeros((640, 512), np.float32)
    wcT_full[:512] = W_c.T
    wcT_full[512] = b_c
    wca_full = np.zeros((640, 512), np.float32)
    wca_full[:512] = 0.5 * (W_c.T @ W_a)
    wca_full[512] = 0.5 * (b_c @ W_a)
    i8 = np.eye(8, dtype=np.float32)
    mi8 = np.zeros((128, 8), np.float32)
    mi8[:8, :8] = np.eye(8)
    mi8[8, :] = 1.0
    b0c = np.tile(b0s[None, :], (128, 1)).astype(np.float32)
    b1r = np.tile(b1s[None, :], (8, 1))
    use_b0 = bool(np.any(b0 != 0))
    use_b1 = bool(np.any(b1 != 0))
    sb_ba = (nf @ W_c.T + b_c) @ b_a      # per-node b_a fold for scores

    maps = []
    for k in range(NCORES):
        sel = (bidx >= k * BL) & (bidx < (k + 1) * BL)
        nodes = np.nonzero(sel)[0]
        cnt = len(nodes)
        nfT_full = np.zeros((640, n_pad), np.float32)
        nfT_full[:512, :cnt] = nf[nodes].T
        nfT_full[512, :cnt] = 1.0
        lb = bidx[nodes] - k * BL
        msk = np.zeros((128, n_pad), np.float32)
        msk[8, :] = NEG
        msk[8, :cnt] += sb_ba[nodes]
        msk[lb, np.arange(cnt)] = -NEG
        e = emb[caps[k * BL:(k + 1) * BL]]             # [8, T, E]
        embT_full = np.ascontiguousarray(e.transpose(2, 1, 0).reshape(E, POS))
        wfc = np.zeros((VPAD, H), np.float32)
        wfc[:VSH] = 0.5 * W_fc[k * VSH:(k + 1) * VSH]
        wfcb = blocks(np.ascontiguousarray(wfc.T))     # [128, 4, VPAD]
        m = {
            "nfT": blocks(nfT_full).astype(bf),
            "wcT": blocks(wcT_full).astype(bf),
            "wcaT": blocks(wca_full).astype(bf),
            "msk": msk.astype(bf), "mi8": mi8.astype(bf),
            "i8b": i8.astype(bf),
            "embT": blocks(embT_full).astype(bf),
            "wembT": blocks(wemb).astype(bf), "b0c": b0c,
            "w0T": blocks(w0).astype(bf), "w1T": blocks(w1).astype(bf),
            "b1r": b1r.astype(bf),
            "gfT": blocks(np.ascontiguousarray(
                2.0 * gf[k * BL:(k + 1) * BL].T)).astype(bf),
            "wfcb": wfcb.astype(bf),
        }
        maps.append(m)
    return maps, use_b0, use_b1


def kernel(**inputs) -> np.ndarray:
    from concourse.bass_utils import run_bass_kernel_spmd

    bidx = np.asarray(inputs["batch_idx"]).astype(np.int64)
    counts = np.bincount(bidx // BL, minlength=NCORES)
    n_pad = max(256, int(math.ceil(counts.max() / 128.0)) * 128)
    maps, use_b0, use_b1 = _prep(inputs, n_pad)
    key = (n_pad, use_b0, use_b1)
    if key not in _COMPILED:
        _COMPILED[key] = _build(n_pad, use_b0, use_b1)
    res = run_bass_kernel_spmd(_COMPILED[key], maps,
                               core_ids=list(range(NCORES)))
    b_fc = np.asarray(inputs["b_fc"], np.float32)
    out = np.empty((B, T, V), np.float32)
    for k in range(NCORES):
        o = np.asarray(res.results[k]["out"]).astype(np.float32)
        # [ch, pc, c4, dt, b, v] -> [pc, c4, b, ch, dt, v]
        o = o.reshape(NCH, 2, 4, 4, 8, VPAD).transpose(1, 2, 4, 0, 3, 5)
        out[:, :, k * VSH:(k + 1) * VSH] = o.reshape(B, T, VPAD)[:, :, :VSH]
    if np.any(b_fc != 0):
        out += b_fc[None, None, :]
    return out
